# revision 7
# baseline (speedup 1.0000x reference)
"""Single-head causal self-attention on 8 TRN2 NeuronCores — v6.

Problem: B=8, T=2048, C=1024 fp32.
  q = x @ Wq.T + bq ; k = x @ Wk.T + bk ; v = x @ Wv.T + bv
  att = softmax(causal_mask(q @ k.T / sqrt(C)))
  out = att @ v

Sharding: data-parallel over batch — core b owns batch element b.

Key structure (v2-v5): scores are computed TRANSPOSED (S^T[s,t] with the
key index s on partitions) so the exp'd tiles are already in lhsT
orientation for att@V — no PE transposes. The rank-1 softmax bias
correction w[s] (= scale * bq^T Wk x_s; row-constant terms cancel in
softmax) is a per-partition ACT bias folded into the exp. Row sums l[t]
come from N=1 ones-column matmuls sharing the att@V stationary operand.
att@V runs as two sequential per-block passes so each block's
normalization tail (on DVE) overlaps the next stretch of matmuls.

v7 (v6 +merged output stores): the kt projection (kt = x @ (Wk^T Wq)/sqrt(C), the biggest single
matmul block) runs in fp8-e4m3 DoubleRow mode: channel-pair packed
operands contract 256 channels per matmul at the same 213ns/MM
LDW-bound stream rate, halving the matmul count (54.6us -> ~27us of PE).
Host ships x/4 and m*256 in fp8 (balanced so both use fp8's normal
range); the ACT copy rescales by 1/64. Scores, v, and att@V stay bf16;
measured end-to-end rel err 0.0156 vs the 2e-2 gate.

The v-projection bias passes through the attention average unchanged
(attention weights sum to 1), so bv is added once at the end.
"""

import numpy as np
import ml_dtypes

B, T, C = 8, 2048, 1024
P = 128            # partitions
C8 = C // P        # contraction chunks (8)
G4 = C // 256      # fp8 DoubleRow channel-pair groups (4)
NT = T // P        # 16 key blocks of 128
ST = 512           # t-tile width for projections
NST = T // ST      # 4 t-chunks across full T
SCALE = 1.0 / np.sqrt(C)
LAM_X = 0.125      # x fp8 pre-scale
LAM_M = 512.0      # m fp8 pre-scale
UNSCALE = 1.0 / (LAM_X * LAM_M)

BF16 = ml_dtypes.bfloat16
F8E4 = ml_dtypes.float8_e4m3fn


def build_nc():
    import concourse.tile as tile
    from concourse import bacc, mybir

    f32 = mybir.dt.float32
    bf16 = mybir.dt.bfloat16
    f8 = mybir.dt.float8e4

    nc = bacc.Bacc()

    xt = nc.declare_dram_parameter("xt", [P, NST, C8 * ST], bf16, isOutput=False)
    x8 = nc.declare_dram_parameter("x8", [P, NST, C8 * ST], f8, isOutput=False)
    m8 = nc.declare_dram_parameter("m8", [P, G4 * 2 * C], f8, isOutput=False)
    wvt = nc.declare_dram_parameter("wvt", [P, 2, C8 * 512], bf16, isOutput=False)
    wvec = nc.declare_dram_parameter("wvec", [P, NT], f32, isOutput=False)
    ones1 = nc.declare_dram_parameter("ones1", [P, 1], bf16, isOutput=False)
    bvb = nc.declare_dram_parameter("bvb", [P, C], f32, isOutput=False)
    maskd = nc.declare_dram_parameter("maskd", [P, P], bf16, isOutput=False)
    out = nc.declare_dram_parameter("out", [T, C], bf16, isOutput=True)

    with tile.TileContext(nc) as tc:
        import contextlib
        ctx = contextlib.ExitStack()
        with ctx:
            consts = ctx.enter_context(tc.tile_pool(name="consts", bufs=1))
            work = ctx.enter_context(tc.tile_pool(name="work", bufs=1))
            ppool = ctx.enter_context(tc.tile_pool(name="ppool", bufs=18))
            lpool = ctx.enter_context(tc.tile_pool(name="lpool", bufs=4))
            opool = ctx.enter_context(tc.tile_pool(name="opool", bufs=4))
            psum = ctx.enter_context(tc.tile_pool(name="psum", bufs=1, space="PSUM"))

            # startup DMAs in consumption order: kt-DR needs x8 tt0 + m8,
            # then the v projection needs xt tt0 + wv (ft-major halves).
            x8_sb = work.tile([P, NST, C8 * ST], f8, tag="x8")
            nc.sync.dma_start(out=x8_sb[:, 0, :], in_=x8[:, 0, :])
            m8_sb = work.tile([P, G4 * 2 * C], f8, tag="m8")
            g8 = 2 * C
            for g in range(G4):
                nc.sync.dma_start(
                    out=m8_sb[:, g * g8:(g + 1) * g8],
                    in_=m8[:, g * g8:(g + 1) * g8],
                )
            xt_sb = work.tile([P, NST, C8 * ST], bf16, tag="xt")
            nc.sync.dma_start(out=xt_sb[:, 0, :], in_=xt[:, 0, :])
            wv_sb = work.tile([P, 2, C8 * 512], bf16, tag="wv")
            for ft in range(2):
                nc.sync.dma_start(out=wv_sb[:, ft, :], in_=wvt[:, ft, :])
            for tc_ in range(1, NST):
                nc.sync.dma_start(out=xt_sb[:, tc_, :], in_=xt[:, tc_, :])
                nc.sync.dma_start(out=x8_sb[:, tc_, :], in_=x8[:, tc_, :])
            ones_sb = consts.tile([P, 1], bf16, tag="ones1")
            nc.sync.dma_start(out=ones_sb, in_=ones1[:, :])
            maskd_sb = consts.tile([P, P], bf16, tag="maskd")
            nc.sync.dma_start(out=maskd_sb, in_=maskd[:, :])
            bvb_sb = consts.tile([P, C], f32, tag="bvb")
            nc.sync.dma_start(out=bvb_sb, in_=bvb[:, :])
            w_sb = consts.tile([P, NT], f32, tag="w_sb")
            nc.sync.dma_start(out=w_sb, in_=wvec[:, :])

            # HAM warm-up: dummy matmuls on a memset tile (no DMA dependency)
            # keep the PE's activity window open until the first real operands
            # land, so the real matmuls start at 2.4 GHz.
            warm_sb = consts.tile([P, P], bf16, tag="warm_sb")
            nc.vector.memset(warm_sb, 1.0)
            warm = psum.tile([P, 256], f32, tag="ps_b", bufs=2, name="warm")
            for _ in range(100):
                nc.tensor.matmul(warm[:, 0:64], warm_sb, warm_sb[:, 0:64],
                                 start=True, stop=True)

            xt_v = xt_sb.rearrange("p tc (c8 t) -> p tc c8 t", t=ST)
            x8_v = x8_sb.rearrange("p tc (c8 t) -> p tc c8 t", t=ST)
            m8_v = m8_sb.rearrange("p (g two f) -> p g two f", g=G4, two=2)
            wv_v = wv_sb.rearrange("p ft (c8 f) -> p ft c8 f", f=512)

            kt_sb = work.tile([P, C8, T], bf16, tag="kt")
            v_sb = work.tile([P, NT, C], bf16, tag="v")

            # ---- phase 1: projections, tt-outer ----
            # kt[c_out, s] = sum_c m[c, c_out] x[s, c]  (fp8 DoubleRow)
            # v[s, f] (bf16), and w[s] = sum_c m2[c] x[s, c]
            for tt in range(NST):
                # f8-pairs with g-outer accumulation: the first matmuls
                # need only the g=0 slab of m8 (starts ~4us earlier), and
                # two spare ps_a buffers keep the ACT drains off the
                # critical path.
                for f8p in range(C8 // 2):
                    pss = [
                        psum.tile([P, ST], f32, tag="ps_a", bufs=4,
                                  name="ps_kt")
                        for _ in range(2)
                    ]
                    for g in range(G4):
                        for k in range(2):
                            f8 = 2 * f8p + k
                            nc.tensor.matmul(
                                pss[k],
                                m8_v[:, g, :, f8 * P:(f8 + 1) * P],
                                x8_v[:, tt, 2 * g:2 * g + 2, :],
                                start=(g == 0),
                                stop=(g == G4 - 1),
                                perf_mode=mybir.MatmulPerfMode.DoubleRow,
                            )
                    for k in range(2):
                        nc.scalar.activation(
                            out=kt_sb[:, 2 * f8p + k, tt * ST:(tt + 1) * ST],
                            in_=pss[k],
                            func=mybir.ActivationFunctionType.Identity,
                            scale=UNSCALE,
                        )
                for ft in range(2):
                    for s4 in range(4):
                        s16 = tt * 4 + s4
                        psv = psum.tile([P, ST], f32, tag="ps_a", bufs=4,
                                        name="ps_v")
                        for c8 in range(C8):
                            lhsT = xt_v[:, tt, c8, s4 * P:(s4 + 1) * P]
                            nc.tensor.matmul(
                                psv, lhsT, wv_v[:, ft, c8, :],
                                start=(c8 == 0), stop=(c8 == C8 - 1),
                            )
                        nc.vector.tensor_copy(
                            out=v_sb[:, s16, ft * 512:(ft + 1) * 512], in_=psv
                        )

            # ---- phase 2: attention, two 128-row query blocks per pair ----
            for i in range(NT // 2):
                tt, off = i // 2, (i % 2) * 256
                nch = 2 * i + 2          # s-chunks 0..2i+1

                def emit_scores(j):
                    # chunk 2i+1 only feeds block 2i+1 (cols 128:256)
                    o, w = (0, 256) if j < 2 * i + 1 else (P, P)
                    ps_s = psum.tile([P, 256], f32, tag="ps_b", bufs=2,
                                     name="ps_s")
                    for c8 in range(C8):
                        nc.tensor.matmul(
                            ps_s[:, o:o + w],
                            kt_sb[:, c8, j * P:(j + 1) * P],
                            xt_v[:, tt, c8, off + o:off + o + w],
                            start=(c8 == 0),
                            stop=(c8 == C8 - 1),
                        )
                    pT = ppool.tile([P, 256], bf16, tag="pT", bufs=18, name="pT")
                    nc.scalar.activation(
                        out=pT[:, o:o + w], in_=ps_s[:, o:o + w],
                        func=mybir.ActivationFunctionType.Exp,
                        bias=w_sb[:, j:j + 1],
                        scale=1.0,
                    )
                    if j == 2 * i:
                        nc.vector.tensor_mul(pT[:, 0:P], pT[:, 0:P], maskd_sb)
                    elif j == 2 * i + 1:
                        nc.vector.tensor_mul(pT[:, P:2 * P], pT[:, P:2 * P],
                                             maskd_sb)
                    return pT

                # one l-accumulator per block, in separate PSUM banks: a
                # start=True clear touches the whole bank, and a start=False
                # group would inherit uninitialized has_written state on a
                # fresh NEFF load.
                ps_l0 = psum.tile([P, 1], f32, tag="ps_w", bufs=1, name="ps_l0")
                ps_l1 = psum.tile([P, 1], f32, tag="ps_lb", bufs=1, name="ps_l1")

                def emit_tail(blk, ps_pair, ps_l):
                    # out = ps_o * (1/l) + bv, all on DVE so the ACT queue
                    # stays free for the next pair's exp tiles. split=True
                    # (last block only) streams quarter-width DVE->DMA so the
                    # final store isn't one serial 512KB chain.
                    rl = lpool.tile([P, 1], f32, name="rl", tag="rl")
                    nc.vector.reciprocal(out=rl, in_=ps_l)
                    o_sb = opool.tile([P, 1024], bf16, name="o_sb",
                                      tag="o_sb", bufs=4)
                    for ft in range(2):
                        nc.vector.scalar_tensor_tensor(
                            out=o_sb[:, ft * 512:(ft + 1) * 512],
                            in0=ps_pair[ft], scalar=rl,
                            in1=bvb_sb[:, ft * 512:(ft + 1) * 512],
                            op0=mybir.AluOpType.mult,
                            op1=mybir.AluOpType.add,
                        )
                    nc.sync.dma_start(
                        out=out[blk * P:(blk + 1) * P, :], in_=o_sb,
                    )

                # block 2i: att@V streams behind the score tiles
                ps_o0 = [
                    psum.tile([P, 512], f32, tag="ps_a", bufs=4, name="ps_o0")
                    for _ in range(2)
                ]
                pTs = [emit_scores(0)]
                for j in range(nch):
                    if j + 1 < nch:
                        pTs.append(emit_scores(j + 1))
                    if j <= 2 * i:
                        nc.tensor.matmul(ps_o0[0], pTs[j][:, 0:P],
                                         v_sb[:, j, 0:512],
                                         start=(j == 0), stop=(j == 2 * i))
                        nc.tensor.matmul(ps_o0[1], pTs[j][:, 0:P],
                                         v_sb[:, j, 512:1024],
                                         start=(j == 0), stop=(j == 2 * i))
                        nc.tensor.matmul(ps_l0, pTs[j][:, 0:P], ones_sb,
                                         start=(j == 0), stop=(j == 2 * i))
                emit_tail(2 * i, ps_o0, ps_l0)

                # block 2i+1: second pass over the retained p^T tiles; its
                # tail (and block 2i's) overlap this pass / the next pair's
                # scores instead of stalling the PE.
                ps_o1 = [
                    psum.tile([P, 512], f32, tag="ps_a", bufs=4, name="ps_o1")
                    for _ in range(2)
                ]
                if i < NT // 2 - 1:
                    for j in range(nch):
                        nc.tensor.matmul(ps_o1[0], pTs[j][:, P:2 * P],
                                         v_sb[:, j, 0:512],
                                         start=(j == 0), stop=(j == nch - 1))
                        nc.tensor.matmul(ps_o1[1], pTs[j][:, P:2 * P],
                                         v_sb[:, j, 512:1024],
                                         start=(j == 0), stop=(j == nch - 1))
                        nc.tensor.matmul(ps_l1, pTs[j][:, P:2 * P], ones_sb,
                                         start=(j == 0), stop=(j == nch - 1))
                    emit_tail(2 * i + 1, ps_o1, ps_l1)
                else:
                    # final block: ft-sequential att@V so ft0's normalize and
                    # store hide under ft1's matmuls — shortens the kernel's
                    # last serial chain.
                    blk = 2 * i + 1
                    rl1 = lpool.tile([P, 1], f32, name="rl", tag="rl")
                    o_sb1 = opool.tile([P, 1024], bf16, name="o_sb",
                                       tag="o_sb", bufs=4)
                    for ft in range(2):
                        for j in range(nch):
                            nc.tensor.matmul(
                                ps_o1[ft], pTs[j][:, P:2 * P],
                                v_sb[:, j, ft * 512:(ft + 1) * 512],
                                start=(j == 0), stop=(j == nch - 1))
                            if ft == 0:
                                nc.tensor.matmul(ps_l1, pTs[j][:, P:2 * P],
                                                 ones_sb, start=(j == 0),
                                                 stop=(j == nch - 1))
                        if ft == 0:
                            nc.vector.reciprocal(out=rl1, in_=ps_l1)
                        for q in range(2):
                            lo = ft * 512 + q * 256
                            nc.vector.scalar_tensor_tensor(
                                out=o_sb1[:, lo:lo + 256],
                                in0=ps_o1[ft][:, q * 256:q * 256 + 256],
                                scalar=rl1,
                                in1=bvb_sb[:, lo:lo + 256],
                                op0=mybir.AluOpType.mult,
                                op1=mybir.AluOpType.add,
                            )
                            # alternate issue queues (scalar/gpsimd are idle
                            # here) so the kernel's last stores dispatch in
                            # parallel instead of serializing on sync.
                            eng = (nc.sync, nc.gpsimd, nc.scalar,
                                   nc.sync)[2 * ft + q]
                            eng.dma_start(
                                out=out[blk * P:(blk + 1) * P, lo:lo + 256],
                                in_=o_sb1[:, lo:lo + 256],
                            )

    nc.finalize()
    return nc


def make_in_maps(x, Wq, bq, Wk, bk, Wv, bv):
    """Host-side prep: per-core shards + replicated constants, laid out
    partition-major so each DMA is one contiguous run per partition."""
    x = np.asarray(x, dtype=np.float32)
    wvt = np.ascontiguousarray(np.asarray(Wv, np.float32).T).astype(BF16)

    wk32 = np.asarray(Wk, np.float32)
    wq32 = np.asarray(Wq, np.float32)
    m_mat = (wk32.T @ wq32) * SCALE                  # [c_in, c_out] fp32

    # fp8 DoubleRow weights: m8[g, p, i, c_out] = m[256g + 128i + p, c_out]
    # pre-scaled by LAM_M; layout [p, g*2*C] contiguous per partition.
    m8q = (m_mat * LAM_M).astype(F8E4)
    m84 = np.ascontiguousarray(
        m8q.reshape(G4, 2, P, C).transpose(2, 0, 1, 3).reshape(P, G4 * 2 * C)
    )
    m2v = ((wk32.T @ np.asarray(bq, np.float32)) * SCALE).astype(BF16)
    m2f = m2v.astype(np.float32)
    ones1 = np.ones((P, 1), dtype=BF16)
    # [c, f] -> [p, ft, c8*512] (ft-major halves so the v projection can
    # start on the first 1MB of weights)
    wv4 = np.ascontiguousarray(
        wvt.reshape(C8, P, 2, 512).transpose(1, 2, 0, 3).reshape(P, 2, C8 * 512)
    )

    bvb = np.tile(np.asarray(bv, np.float32)[None, :], (P, 1))

    # maskd[p, c] = 1 if p <= c (valid key s=base+p for query t=base+c)
    maskd = np.triu(np.ones((P, P), dtype=np.float32)).astype(BF16)

    in_maps = []
    for b in range(B):
        xtb = np.ascontiguousarray(x[b].T)
        # [c, t] -> [p, tc, c8*t]
        def chunked(a):
            return np.ascontiguousarray(
                a.reshape(C8, P, NST, ST).transpose(1, 2, 0, 3)
                .reshape(P, NST, C8 * ST)
            )
        xt4 = chunked(xtb.astype(BF16))
        x84 = chunked((xtb * LAM_X).astype(F8E4))
        # w[s] = scale * bq^T Wk x_s, from the same bf16 operands the
        # device would use; [T] -> [p, block]
        wv_host = (xtb.astype(BF16).astype(np.float32).T @ m2f)
        wvh = np.ascontiguousarray(wv_host.reshape(NT, P).T.astype(np.float32))
        in_maps.append({
            "xt": xt4, "x8": x84, "m8": m84, "wvt": wv4, "wvec": wvh,
            "ones1": ones1, "bvb": bvb, "maskd": maskd,
        })
    return in_maps


_CACHED_NC = None


def kernel(x, Wq, bq, Wk, bk, Wv, bv):
    global _CACHED_NC
    from concourse.bass_utils import run_bass_kernel_spmd

    if _CACHED_NC is None:
        _CACHED_NC = build_nc()
    in_maps = make_in_maps(x, Wq, bq, Wk, bk, Wv, bv)
    res = run_bass_kernel_spmd(_CACHED_NC, in_maps, core_ids=list(range(B)))
    return np.stack([res.results[b]["out"] for b in range(B)]).astype(np.float32)
